# revision 6
# baseline (speedup 1.0000x reference)
"""Sparse-attention Bass kernel for 8 TRN2 NeuronCores (Schraudolph split-route).

Sharding: query-row parallel. Core c owns query rows [c*512, (c+1)*512) of
both batch elements. K/V computed redundantly per core (no collectives).

The softmax exp is the bottleneck (67M elements/core must exit PSUM through
ACT or DVE at ~1 elem/cycle/lane). Two routes, statically interleaved:

  RC (~5/8 of score tiles): the PE pre-adds maskbias into the scores PSUM
     via an identity matmul (masked lanes get -30080 -> exp underflows to 0),
     then ONE ACT Exp instruction exits PSUM->SBUF bf16. Zero DVE cost.
  RB (~3/8): ONE DVE tensor_tensor (scores_f32 + maskbias_bf16 -> int16,
     round-to-nearest) computes a Schraudolph bit-trick exp: the int16 IS
     the bf16 bit pattern of exp(s)*2.41 (Q pre-scaled by 128*log2(e)*scale
     on the host). A second tensor (em2 = em1+64 bits) and a second
     accumulating AV matmul implement a two-point correction that cancels
     the piecewise-linear error (CV 1.78% -> 0.56%).

Rowsums come from M=1 ones-matmuls packed in tile_position col-bands, with
per-route ones weights (1.0 vs rho=2.414) so both routes carry the same
scale; the 2pt rowsum uses the constant-ratio approximation (row-to-row
variance ~0.04%).
"""

import numpy as np
from contextlib import ExitStack

import concourse.bass as bass
import concourse.tile as tile
from concourse import bacc, mybir
from concourse.bass_utils import run_bass_kernel_spmd

BF16 = mybir.dt.bfloat16
F32 = mybir.dt.float32
I16 = mybir.dt.int16
NPBF16 = mybir.dt.np(BF16)

B, N, DIM, H, D = 2, 4096, 512, 16, 32
NCORES = 8
NQ = N // NCORES
G = 4                       # head groups (4 heads each)
HG = H // G
JB = N // 128               # key blocks per batch elem (32)
SCALE = float(D) ** -0.5

# Schraudolph constants
A_SCH = 128.0 / float(np.log(2.0))      # bits per natural-log unit
BC = 127 * 128                          # 16256, bf16 exponent bias in bits
MASK_NEG = -30080.0                     # masked maskbias value (exact bf16)
D2PT = 64                               # two-point offset (bits)
# mean ratios of the approximations vs true exp (computed offline)
MU1 = 1.040683                          # solo em1 mean ratio
MU2 = 2.512434                          # em1 + em2 mean ratio (w=1, D=64)
RHO = MU2 / MU1                         # RB rowsum ones weight
LN2_128 = float(np.log(2.0)) / 128.0
# ACT exp bias: cancel the +BC carried in maskbias, add ln(MU2) to match RB
ACT_BIAS = -BC * LN2_128 + float(np.log(MU2))

# Route pattern: tile t is RB iff (t*3) % 8 < 3  -> 3/8 RB, 5/8 RC
def is_rb(t):
    return (t * 3) % 8 < 3

_CACHE = {}


def build_nc():
    nc = bacc.Bacc("TRN2", target_bir_lowering=False, debug=False)

    batT = nc.declare_dram_parameter("batt", [B, DIM, N], BF16, isOutput=False)
    qrT = nc.declare_dram_parameter("qrt", [B, DIM, NQ], BF16, isOutput=False)
    wqkv = nc.declare_dram_parameter("wqkv", [DIM, 3 * DIM], BF16, isOutput=False)
    wproj = nc.declare_dram_parameter("wproj", [DIM, DIM], BF16, isOutput=False)
    maskb = nc.declare_dram_parameter("maskb", [N, NQ], BF16, isOutput=False)
    ident = nc.declare_dram_parameter("ident", [128, 128], BF16, isOutput=False)
    out = nc.declare_dram_parameter("out", [B, NQ, DIM], F32, isOutput=True)

    Exp = mybir.ActivationFunctionType.Exp
    ADD = mybir.AluOpType.add

    with tile.TileContext(nc) as tc, ExitStack() as ctx:
        persist = ctx.enter_context(tc.tile_pool(name="persist", bufs=1))
        bpool = ctx.enter_context(tc.tile_pool(name="bpool", bufs=1))
        ktpool = ctx.enter_context(tc.tile_pool(name="ktpool", bufs=2))
        xpool = ctx.enter_context(tc.tile_pool(name="xpool", bufs=8))
        empool = ctx.enter_context(tc.tile_pool(name="empool", bufs=4))
        epool = ctx.enter_context(tc.tile_pool(name="epool", bufs=4))
        npool = ctx.enter_context(tc.tile_pool(name="npool", bufs=2))
        outp = ctx.enter_context(tc.tile_pool(name="outp", bufs=2))
        # PSUM: sc 3x2 banks + av 1 + rs 1 = 8
        scps = ctx.enter_context(tc.tile_pool(name="scps", bufs=3, space="PSUM"))
        avps = ctx.enter_context(tc.tile_pool(name="avps", bufs=1, space="PSUM"))
        rsps = ctx.enter_context(tc.tile_pool(name="rsps", bufs=1, space="PSUM"))

        # ---- persistent loads -------------------------------------------
        wq_sb = []
        for k in range(4):
            t = persist.tile([128, 3 * DIM], BF16, tag=f"wqkv{k}")
            nc.sync.dma_start(out=t, in_=wqkv[k * 128:(k + 1) * 128, :])
            wq_sb.append(t)
        wp_sb = []
        for k in range(4):
            t = persist.tile([128, DIM], BF16, tag=f"wproj{k}")
            nc.sync.dma_start(out=t, in_=wproj[k * 128:(k + 1) * 128, :])
            wp_sb.append(t)
        mb_sb = []
        for jb in range(JB):
            t = persist.tile([128, NQ], BF16, tag=f"mb{jb}")
            nc.sync.dma_start(out=t, in_=maskb[jb * 128:(jb + 1) * 128, :])
            mb_sb.append(t)
        id_sb = persist.tile([128, 128], BF16, tag="ident")
        nc.sync.dma_start(out=id_sb, in_=ident[:, :])
        # ones weights for rowsum matmuls: cols 0..3 -> 1.0 (RC), 4..7 -> RHO (RB)
        ones_sb = persist.tile([128, 8], BF16, tag="ones")
        nc.vector.memset(ones_sb[:, 0:4], 1.0)
        nc.vector.memset(ones_sb[:, 4:8], RHO)
        actbias = persist.tile([128, 1], F32, tag="actbias")
        nc.vector.memset(actbias, ACT_BIAS)

        def mrep(mb_t):
            return bass.AP(
                tensor=mb_t.tensor, offset=mb_t.offset,
                ap=[mb_t.ap[0], [0, 2], [1, NQ]],
            )

        tile_ctr = [0]

        for b in range(B):
            # per-b SBUF
            qrT_sb = []
            for k in range(4):
                t = bpool.tile([128, NQ], BF16, tag=f"qrT{k}")
                nc.sync.dma_start(out=t, in_=qrT[b, k * 128:(k + 1) * 128, :])
                qrT_sb.append(t)
            v_sb = bpool.tile([128, JB * 512], BF16, tag="vall")
            qt_sb = [bpool.tile([128, NQ], BF16, tag=f"qt{g}", name=f"qt{g}")
                     for g in range(G)]

            # batT slices streamed: jc in 0..7, each [4k][128, 512]
            bat_tiles = {}

            def load_bat(jc):
                tl = []
                for k in range(4):
                    t = xpool.tile([128, 512], BF16, tag=f"bat{k}")
                    nc.sync.dma_start(
                        out=t, in_=batT[b, k * 128:(k + 1) * 128,
                                        jc * 512:(jc + 1) * 512])
                    tl.append(t)
                bat_tiles[jc] = tl

            def make_kt_pair(kt_t, g, jc2):
                # two adjacent 512-chunks of K^T(g) -> one [128,1024] psum
                # tile -> one ACT copy into kt_t[:, jc2*1024 : +1024]
                ps = scps.tile([128, 1024], F32, tag="sc")
                for half in range(2):
                    jc = jc2 * 2 + half
                    bt = bat_tiles[jc]
                    for k in range(4):
                        nc.tensor.matmul(
                            ps[:, half * 512:(half + 1) * 512],
                            wq_sb[k][:, DIM + 128 * g: DIM + 128 * (g + 1)],
                            bt[k],
                            start=(k == 0), stop=(k == 3),
                        )
                nc.scalar.copy(kt_t[:, jc2 * 1024:(jc2 + 1) * 1024], ps)

            def make_v_pair(nb2):
                # two adjacent 128-row V blocks -> [128,1024] psum -> v_sb
                ps = scps.tile([128, 1024], F32, tag="sc")
                for half in range(2):
                    nb = nb2 * 2 + half
                    jc = nb // 4
                    bt = bat_tiles[jc]
                    off = (nb % 4) * 128
                    for k in range(4):
                        nc.tensor.matmul(
                            ps[:, half * 512:(half + 1) * 512],
                            bt[k][:, off:off + 128],
                            wq_sb[k][:, 2 * DIM: 3 * DIM],
                            start=(k == 0), stop=(k == 3),
                        )
                nc.scalar.copy(v_sb[:, nb2 * 1024:(nb2 + 1) * 1024], ps)

            def make_qt(g):
                ps = scps.tile([128, 1024], F32, tag="sc")
                for k in range(4):
                    nc.tensor.matmul(
                        ps[:, 0:512],
                        wq_sb[k][:, 128 * g: 128 * (g + 1)],
                        qrT_sb[k],
                        start=(k == 0), stop=(k == 3),
                    )
                nc.scalar.copy(qt_sb[g], ps[:, 0:512])

            # ---- preamble: batT jc=0..7 streamed in, kt[0], qt[0], v 0..7
            kt_cur = ktpool.tile([128, N], BF16, tag="kt")
            for jc in range(8):
                load_bat(jc)
            make_qt(0)
            for jc2 in range(4):
                make_kt_pair(kt_cur, 0, jc2)
            for nb2 in range(4):
                make_v_pair(nb2)

            # deferred work lists per g: list of callables
            kt_next = [None]
            def defer_for_g(g):
                work = []
                if g == 0:
                    for nb2 in range(4, 16):
                        work.append(lambda nb2=nb2: make_v_pair(nb2))
                if g < G - 1:
                    nk = ktpool.tile([128, N], BF16, tag="kt")
                    kt_next[0] = nk
                    for jc2 in range(4):
                        work.append(lambda nk=nk, g1=g + 1, jc2=jc2:
                                    make_kt_pair(nk, g1, jc2))
                    work.append(lambda g1=g + 1: make_qt(g1))
                return work

            pre_sb = [bpool.tile([128, NQ], BF16, tag=f"pre{g}", name=f"pre{g}")
                      for g in range(G)]

            for g in range(G):
                work = defer_for_g(g)
                wi = 0
                av = avps.tile([128, NQ], F32, tag="av")
                rs = rsps.tile([128, NQ], F32, tag="rs")
                av_started = [False] * HG
                rs_started = [False] * HG

                for jb in range(JB):
                    # interleave deferred QKV work (g0 has 17 items: 1/iter)
                    if wi < len(work) and (g == 0 or jb % 2 == 0):
                        work[wi](); wi += 1

                    mb_t = mb_sb[jb]
                    em_halves = []  # (ap_bf16, is2pt) per head r
                    for pair in range(2):     # heads (0,1) then (2,3)
                        t_idx = tile_ctr[0]; tile_ctr[0] += 1
                        rb = is_rb(t_idx)
                        sc = scps.tile([128, 1024], F32, tag="sc")
                        for r2 in range(2):
                            r = pair * 2 + r2
                            nc.tensor.matmul(
                                sc[:, r2 * 512:(r2 + 1) * 512],
                                kt_cur[32 * r:32 * r + 32, jb * 128:(jb + 1) * 128],
                                qt_sb[g][32 * r:32 * r + 32, :],
                                start=True, stop=rb,
                                tile_position=(32 * r, 0),
                            )
                        if rb:
                            em1 = empool.tile([128, 1024], I16, tag="em1")
                            nc.vector.tensor_tensor(em1, sc, mrep(mb_t), ADD)
                            em2 = empool.tile([128, 1024], I16, tag="em2")
                            nc.vector.tensor_scalar_add(em2, em1, float(D2PT))
                            e1b = em1.bitcast(BF16)
                            e2b = em2.bitcast(BF16)
                            for r2 in range(2):
                                em_halves.append(
                                    ((e1b[:, r2 * 512:(r2 + 1) * 512],
                                      e2b[:, r2 * 512:(r2 + 1) * 512]), True))
                        else:
                            # mask via identity-matmul accumulation, then exp
                            for r2 in range(2):
                                nc.tensor.matmul(
                                    sc[:, r2 * 512:(r2 + 1) * 512],
                                    id_sb, mb_t,
                                    start=False, stop=True,
                                )
                            e = epool.tile([128, 1024], BF16, tag="e")
                            nc.scalar.activation(e, sc, Exp,
                                                 scale=LN2_128, bias=actbias)
                            for r2 in range(2):
                                em_halves.append(
                                    ((e[:, r2 * 512:(r2 + 1) * 512], None), False))

                    # AV wave 1 (all 4 heads) + AV wave 2 (RB heads only)
                    vsl = v_sb[:, jb * 512:(jb + 1) * 512]
                    last_jb = (jb == JB - 1)
                    for r in range(HG):
                        (h1, h2), rb = em_halves[r]
                        hh = g * HG + r
                        nc.tensor.matmul(
                            av[32 * r:32 * r + 32, :],
                            vsl[:, 32 * hh:32 * hh + 32], h1,
                            start=not av_started[r],
                            stop=last_jb and not rb,
                            tile_position=(0, 32 * r),
                        )
                        av_started[r] = True
                    for r in range(HG):
                        (h1, h2), rb = em_halves[r]
                        if rb:
                            hh = g * HG + r
                            nc.tensor.matmul(
                                av[32 * r:32 * r + 32, :],
                                vsl[:, 32 * hh:32 * hh + 32], h2,
                                start=False, stop=last_jb,
                                tile_position=(0, 32 * r),
                            )
                    # rowsum wave (M=1 per head, col-banded)
                    for r in range(HG):
                        (h1, h2), rb = em_halves[r]
                        nc.tensor.matmul(
                            rs[32 * r:32 * r + 1, :],
                            ones_sb[:, (4 + r if rb else r):(5 + r if rb else r + 1)],
                            h1,
                            start=not rs_started[r], stop=last_jb,
                            tile_position=(0, 32 * r),
                        )
                        rs_started[r] = True

                # ---- normalize group g (baseline-proven pattern) --------
                for r in range(HG):
                    rsr = npool.tile([1, NQ], F32, tag="rsr")
                    nc.vector.tensor_copy(rsr, rs[32 * r:32 * r + 1, :])
                    rcp = npool.tile([1, NQ], F32, tag="rcp")
                    nc.vector.reciprocal_approx_fast(rcp, rsr)
                    rcpb = npool.tile([32, NQ], F32, tag="rcpb")
                    nc.gpsimd.partition_broadcast(rcpb, rcp[0:1, :], channels=32)
                    nc.vector.tensor_mul(
                        pre_sb[g][32 * r:32 * r + 32, :],
                        av[32 * r:32 * r + 32, :],
                        rcpb,
                    )

                if g < G - 1:
                    kt_cur = kt_next[0]

            # ---- output projection --------------------------------------
            for ib in range(NQ // 128):
                ps = rsps.tile([128, NQ], F32, tag="rs")
                for g in range(G):
                    nc.tensor.matmul(
                        ps[:, 0:DIM],
                        pre_sb[g][:, ib * 128:(ib + 1) * 128],
                        wp_sb[g],
                        start=(g == 0), stop=(g == 3),
                    )
                o = outp.tile([128, DIM], F32, tag="o")
                nc.scalar.copy(o, ps[:, 0:DIM])
                nc.sync.dma_start(out=out[b, ib * 128:(ib + 1) * 128, :], in_=o)

    nc.compile()
    return nc


def _prep_inputs(batch, w_qkv, w_proj, custom_mask):
    batch = np.asarray(batch, np.float32)
    w_qkv = np.asarray(w_qkv, np.float32).copy()
    w_qkv[:, :DIM] *= A_SCH * SCALE          # Schraudolph pre-scale on Q
    wqkv_bf = w_qkv.astype(NPBF16)
    wproj_bf = np.asarray(w_proj, np.float32).astype(NPBF16)
    batT = np.ascontiguousarray(batch.transpose(0, 2, 1)).astype(NPBF16)
    m = np.asarray(custom_mask, np.float32)[0, 0]  # [N, N] 0/1
    ident = np.eye(128, dtype=np.float32).astype(NPBF16)
    in_maps = []
    for c in range(NCORES):
        rows = slice(c * NQ, (c + 1) * NQ)
        qrTc = np.ascontiguousarray(batch[:, rows, :].transpose(0, 2, 1)).astype(NPBF16)
        mT = np.ascontiguousarray(m[rows, :].T)   # [N, NQ]
        mb = np.where(mT > 0, float(BC), MASK_NEG).astype(np.float32).astype(NPBF16)
        in_maps.append({
            "batt": batT, "qrt": qrTc, "wqkv": wqkv_bf,
            "wproj": wproj_bf, "maskb": mb, "ident": ident,
        })
    return in_maps


def _run(in_maps, trace=False, **kw):
    if "nc" not in _CACHE:
        _CACHE["nc"] = build_nc()
    return run_bass_kernel_spmd(
        _CACHE["nc"], in_maps, core_ids=list(range(NCORES)), trace=trace, **kw
    )


def kernel(batch, w_qkv, w_proj, custom_mask):
    in_maps = _prep_inputs(batch, w_qkv, w_proj, custom_mask)
    res = _run(in_maps)
    full = np.empty((B, N, DIM), np.float32)
    for c in range(NCORES):
        full[:, c * NQ:(c + 1) * NQ, :] = res.results[c]["out"]
    return full


# revision 9
# speedup vs baseline: 1.2702x; 1.2702x over previous
"""Sparse-attention Bass kernel for 8 TRN2 NeuronCores (Schraudolph split-route).

Sharding: query-row parallel. Core c owns query rows [c*512, (c+1)*512) of
both batch elements. K/V computed redundantly per core (no collectives).

The softmax exp is the bottleneck (67M elements/core must exit PSUM through
ACT or DVE at ~1 elem/cycle/lane). Two routes, statically interleaved:

  RC (~5/8 of score tiles): the PE pre-adds maskbias into the scores PSUM
     via an identity matmul (masked lanes get -30080 -> exp underflows to 0),
     then ONE ACT Exp instruction exits PSUM->SBUF bf16. Zero DVE cost.
  RB (~3/8): ONE DVE tensor_tensor (scores_f32 + maskbias_bf16 -> int16,
     round-to-nearest) computes a Schraudolph bit-trick exp: the int16 IS
     the bf16 bit pattern of exp(s)*2.41 (Q pre-scaled by 128*log2(e)*scale
     on the host). A second tensor (em2 = em1+64 bits) and a second
     accumulating AV matmul implement a two-point correction that cancels
     the piecewise-linear error (CV 1.78% -> 0.56%).

Rowsums come from M=1 ones-matmuls packed in tile_position col-bands, with
per-route ones weights (1.0 vs rho=2.414) so both routes carry the same
scale; the 2pt rowsum uses the constant-ratio approximation (row-to-row
variance ~0.04%).
"""

import numpy as np
from contextlib import ExitStack

import concourse.bass as bass
import concourse.tile as tile
from concourse import bacc, mybir
from concourse.bass_utils import run_bass_kernel_spmd

BF16 = mybir.dt.bfloat16
F32 = mybir.dt.float32
I16 = mybir.dt.int16
NPBF16 = mybir.dt.np(BF16)

B, N, DIM, H, D = 2, 4096, 512, 16, 32
NCORES = 8
NQ = N // NCORES
G = 4                       # head groups (4 heads each)
HG = H // G
JB = N // 128               # key blocks per batch elem (32)
SCALE = float(D) ** -0.5

# Schraudolph constants
A_SCH = 128.0 / float(np.log(2.0))      # bits per natural-log unit
BC = 127 * 128                          # 16256, bf16 exponent bias in bits
MASK_NEG = -30080.0                     # masked maskbias value (exact bf16)
D2PT = 64                               # two-point offset (bits)
# mean ratios of the approximations vs true exp (computed offline)
MU1 = 1.040683                          # solo em1 mean ratio
MU2 = 2.512434                          # em1 + em2 mean ratio (w=1, D=64)
RHO = MU2 / MU1                         # RB rowsum ones weight
LN2_128 = float(np.log(2.0)) / 128.0
# ACT exp bias: cancel the +BC carried in maskbias, add ln(MU2) to match RB
ACT_BIAS = -BC * LN2_128 + float(np.log(MU1))

# Route pattern: tile t is RB iff (t*3) % 8 < 3  -> 3/8 RB, 5/8 RC
def is_rb(t):
    return (t * 3) % 8 < 3

# Among RC tiles, a small subset uses the PE identity-matmul mask
# (relieves DVE); the rest use a DVE mask-multiply.
def is_idmm(t):
    return (t % 7) == 3

_CACHE = {}


def build_nc():
    nc = bacc.Bacc("TRN2", target_bir_lowering=False, debug=False)

    batT = nc.declare_dram_parameter("batt", [B, DIM, N], BF16, isOutput=False)
    qrT = nc.declare_dram_parameter("qrt", [B, DIM, NQ], BF16, isOutput=False)
    wqkv = nc.declare_dram_parameter("wqkv", [DIM, 3 * DIM], BF16, isOutput=False)
    wproj = nc.declare_dram_parameter("wproj", [DIM, DIM], BF16, isOutput=False)
    maskb = nc.declare_dram_parameter("maskb", [N, NQ], BF16, isOutput=False)
    mask01 = nc.declare_dram_parameter("mask01", [N, NQ], BF16, isOutput=False)
    ident = nc.declare_dram_parameter("ident", [128, 128], BF16, isOutput=False)
    out = nc.declare_dram_parameter("out", [B, NQ, DIM], F32, isOutput=True)

    Exp = mybir.ActivationFunctionType.Exp
    ADD = mybir.AluOpType.add

    with tile.TileContext(nc) as tc, ExitStack() as ctx:
        persist = ctx.enter_context(tc.tile_pool(name="persist", bufs=1))
        bpool = ctx.enter_context(tc.tile_pool(name="bpool", bufs=1))
        ktpool = ctx.enter_context(tc.tile_pool(name="ktpool", bufs=2))
        xpool = ctx.enter_context(tc.tile_pool(name="xpool", bufs=8))
        empool = ctx.enter_context(tc.tile_pool(name="empool", bufs=4))
        epool = ctx.enter_context(tc.tile_pool(name="epool", bufs=4))
        npool = ctx.enter_context(tc.tile_pool(name="npool", bufs=1))
        outp = ctx.enter_context(tc.tile_pool(name="outp", bufs=2))
        # PSUM: sc 3x2 banks + av 1 + rs 1 = 8
        scps = ctx.enter_context(tc.tile_pool(name="scps", bufs=3, space="PSUM"))
        avps = ctx.enter_context(tc.tile_pool(name="avps", bufs=1, space="PSUM"))
        rsps = ctx.enter_context(tc.tile_pool(name="rsps", bufs=1, space="PSUM"))

        # ---- persistent loads -------------------------------------------
        wq_sb = []
        for k in range(4):
            t = persist.tile([128, 3 * DIM], BF16, tag=f"wqkv{k}")
            nc.sync.dma_start(out=t, in_=wqkv[k * 128:(k + 1) * 128, :])
            wq_sb.append(t)
        wp_sb = []
        for k in range(4):
            t = persist.tile([128, DIM], BF16, tag=f"wproj{k}")
            nc.sync.dma_start(out=t, in_=wproj[k * 128:(k + 1) * 128, :])
            wp_sb.append(t)
        mb_sb = []
        for jb in range(JB):
            t = persist.tile([128, NQ], BF16, tag=f"mb{jb}")
            nc.sync.dma_start(out=t, in_=maskb[jb * 128:(jb + 1) * 128, :])
            mb_sb.append(t)
        m01_sb = []
        for jb in range(JB):
            t = persist.tile([128, NQ], BF16, tag=f"m01{jb}")
            nc.sync.dma_start(out=t, in_=mask01[jb * 128:(jb + 1) * 128, :])
            m01_sb.append(t)
        id_sb = persist.tile([128, 128], BF16, tag="ident")
        nc.sync.dma_start(out=id_sb, in_=ident[:, :])
        # ones weights for rowsum matmuls: cols 0..3 -> 1.0 (RC), 4..7 -> RHO (RB)
        ones_sb = persist.tile([128, 8], BF16, tag="ones")
        nc.vector.memset(ones_sb, 1.0)
        actbias = persist.tile([128, 1], F32, tag="actbias")
        nc.vector.memset(actbias, ACT_BIAS)
        actbias_p = persist.tile([128, 1], F32, tag="actbiasp")
        nc.vector.memset(actbias_p, float(np.log(MU1)))

        def mrep(mb_t):
            return bass.AP(
                tensor=mb_t.tensor, offset=mb_t.offset,
                ap=[mb_t.ap[0], [0, 2], [1, NQ]],
            )

        tile_ctr = [0]

        for b in range(B):
            # per-b SBUF
            qrT_sb = []
            for k in range(4):
                t = bpool.tile([128, NQ], BF16, tag=f"qrT{k}")
                nc.sync.dma_start(out=t, in_=qrT[b, k * 128:(k + 1) * 128, :])
                qrT_sb.append(t)
            v_sb = bpool.tile([128, JB * 512], BF16, tag="vall")
            qt_sb = [bpool.tile([128, NQ], BF16, tag=f"qt{g}", name=f"qt{g}")
                     for g in range(G)]

            # batT slices streamed: jc in 0..7, each [4k][128, 512]
            bat_tiles = {}

            def load_bat(jc):
                tl = []
                for k in range(4):
                    t = xpool.tile([128, 512], BF16, tag=f"bat{k}")
                    nc.sync.dma_start(
                        out=t, in_=batT[b, k * 128:(k + 1) * 128,
                                        jc * 512:(jc + 1) * 512])
                    tl.append(t)
                bat_tiles[jc] = tl

            def make_kt_pair(kt_t, g, jc2):
                # two adjacent 512-chunks of K^T(g) -> one [128,1024] psum
                # tile -> one ACT copy into kt_t[:, jc2*1024 : +1024]
                ps = scps.tile([128, 1024], F32, tag="sc")
                for half in range(2):
                    jc = jc2 * 2 + half
                    bt = bat_tiles[jc]
                    for k in range(4):
                        nc.tensor.matmul(
                            ps[:, half * 512:(half + 1) * 512],
                            wq_sb[k][:, DIM + 128 * g: DIM + 128 * (g + 1)],
                            bt[k],
                            start=(k == 0), stop=(k == 3),
                        )
                nc.scalar.copy(kt_t[:, jc2 * 1024:(jc2 + 1) * 1024], ps)

            def make_v_pair(nb2):
                # two adjacent 128-row V blocks -> [128,1024] psum -> v_sb
                ps = scps.tile([128, 1024], F32, tag="sc")
                for half in range(2):
                    nb = nb2 * 2 + half
                    jc = nb // 4
                    bt = bat_tiles[jc]
                    off = (nb % 4) * 128
                    for k in range(4):
                        nc.tensor.matmul(
                            ps[:, half * 512:(half + 1) * 512],
                            bt[k][:, off:off + 128],
                            wq_sb[k][:, 2 * DIM: 3 * DIM],
                            start=(k == 0), stop=(k == 3),
                        )
                nc.scalar.copy(v_sb[:, nb2 * 1024:(nb2 + 1) * 1024], ps)

            def make_qt(g):
                ps = scps.tile([128, 1024], F32, tag="sc")
                for k in range(4):
                    nc.tensor.matmul(
                        ps[:, 0:512],
                        wq_sb[k][:, 128 * g: 128 * (g + 1)],
                        qrT_sb[k],
                        start=(k == 0), stop=(k == 3),
                    )
                nc.scalar.copy(qt_sb[g], ps[:, 0:512])

            # ---- preamble: batT jc=0..7 streamed in, kt[0], qt[0], v 0..7
            kt_cur = ktpool.tile([128, N], BF16, tag="kt")
            for jc in range(8):
                load_bat(jc)
            make_qt(0)
            for jc2 in range(4):
                make_kt_pair(kt_cur, 0, jc2)
            for nb2 in range(4):
                make_v_pair(nb2)

            # deferred work lists per g: list of callables
            kt_next = [None]
            def defer_for_g(g):
                work = []
                if g == 0:
                    for nb2 in range(4, 16):
                        work.append(lambda nb2=nb2: make_v_pair(nb2))
                if g < G - 1:
                    nk = ktpool.tile([128, N], BF16, tag="kt")
                    kt_next[0] = nk
                    for jc2 in range(4):
                        work.append(lambda nk=nk, g1=g + 1, jc2=jc2:
                                    make_kt_pair(nk, g1, jc2))
                    work.append(lambda g1=g + 1: make_qt(g1))
                return work

            pre_sb = [bpool.tile([128, NQ], BF16, tag=f"pre{g}", name=f"pre{g}")
                      for g in range(G)]

            for g in range(G):
                work = defer_for_g(g)
                wi = 0
                av = avps.tile([128, NQ], F32, tag="av")
                rs = rsps.tile([128, NQ], F32, tag="rs")
                av_started = [False] * HG
                rs_started = [False] * HG

                def emit_waves(em_halves, wjb, last):
                    vsl = v_sb[:, wjb * 512:(wjb + 1) * 512]
                    for r in range(HG):
                        h1 = em_halves[r]
                        hh = g * HG + r
                        nc.tensor.matmul(
                            av[32 * r:32 * r + 32, :],
                            vsl[:, 32 * hh:32 * hh + 32], h1,
                            start=not av_started[r], stop=last,
                            tile_position=(0, 32 * r),
                        )
                        av_started[r] = True
                    for r in range(HG):
                        h1 = em_halves[r]
                        nc.tensor.matmul(
                            rs[32 * r:32 * r + 1, :],
                            ones_sb[:, r:r + 1], h1,
                            start=not rs_started[r], stop=last,
                            tile_position=(0, 32 * r),
                        )
                        rs_started[r] = True

                pending = None
                for jb in range(JB):
                    # interleave deferred QKV work (g0 has 17 items: 1/iter)
                    if wi < len(work) and (g == 0 or jb % 2 == 0):
                        work[wi](); wi += 1

                    mb_t = mb_sb[jb]
                    em_halves = []
                    for pair in range(2):     # heads (0,1) then (2,3)
                        t_idx = tile_ctr[0]; tile_ctr[0] += 1
                        rb = is_rb(t_idx)
                        idm = (not rb) and is_idmm(t_idx)
                        sc = scps.tile([128, 1024], F32, tag="sc")
                        for r2 in range(2):
                            r = pair * 2 + r2
                            nc.tensor.matmul(
                                sc[:, r2 * 512:(r2 + 1) * 512],
                                kt_cur[32 * r:32 * r + 32, jb * 128:(jb + 1) * 128],
                                qt_sb[g][32 * r:32 * r + 32, :],
                                start=True, stop=not idm,
                                tile_position=(32 * r, 0),
                            )
                        if rb:
                            em1 = empool.tile([128, 1024], I16, tag="em1")
                            nc.vector.tensor_tensor(em1, sc, mrep(mb_t), ADD)
                            e1b = em1.bitcast(BF16)
                            for r2 in range(2):
                                em_halves.append(e1b[:, r2 * 512:(r2 + 1) * 512])
                        elif idm:
                            for r2 in range(2):
                                nc.tensor.matmul(
                                    sc[:, r2 * 512:(r2 + 1) * 512],
                                    id_sb, mb_t,
                                    start=False, stop=True,
                                )
                            e = epool.tile([128, 1024], BF16, tag="e")
                            nc.scalar.activation(e, sc, Exp,
                                                 scale=LN2_128, bias=actbias)
                            for r2 in range(2):
                                em_halves.append(e[:, r2 * 512:(r2 + 1) * 512])
                        else:
                            eraw = epool.tile([128, 1024], BF16, tag="eraw")
                            nc.scalar.activation(eraw, sc, Exp,
                                                 scale=LN2_128, bias=actbias_p)
                            e = epool.tile([128, 1024], BF16, tag="e")
                            nc.vector.tensor_mul(e, eraw, mrep(m01_sb[jb]))
                            for r2 in range(2):
                                em_halves.append(e[:, r2 * 512:(r2 + 1) * 512])

                    if pending is not None:
                        emit_waves(pending[0], pending[1], last=False)
                    pending = (em_halves, jb)
                emit_waves(pending[0], pending[1], last=True)

                # ---- normalize group g (baseline-proven pattern) --------
                for r in range(HG):
                    rsr = npool.tile([1, NQ], F32, tag="rsr")
                    nc.vector.tensor_copy(rsr, rs[32 * r:32 * r + 1, :])
                    rcp = npool.tile([1, NQ], F32, tag="rcp")
                    nc.vector.reciprocal_approx_fast(rcp, rsr)
                    rcpb = npool.tile([32, NQ], F32, tag="rcpb")
                    nc.gpsimd.partition_broadcast(rcpb, rcp[0:1, :], channels=32)
                    nc.vector.tensor_mul(
                        pre_sb[g][32 * r:32 * r + 32, :],
                        av[32 * r:32 * r + 32, :],
                        rcpb,
                    )

                if g < G - 1:
                    kt_cur = kt_next[0]

            # ---- output projection --------------------------------------
            for ib in range(NQ // 128):
                ps = rsps.tile([128, NQ], F32, tag="rs")
                for g in range(G):
                    nc.tensor.matmul(
                        ps[:, 0:DIM],
                        pre_sb[g][:, ib * 128:(ib + 1) * 128],
                        wp_sb[g],
                        start=(g == 0), stop=(g == 3),
                    )
                o = outp.tile([128, DIM], F32, tag="o")
                nc.scalar.copy(o, ps[:, 0:DIM])
                nc.sync.dma_start(out=out[b, ib * 128:(ib + 1) * 128, :], in_=o)

    nc.compile()
    return nc


def _prep_inputs(batch, w_qkv, w_proj, custom_mask):
    batch = np.asarray(batch, np.float32)
    w_qkv = np.asarray(w_qkv, np.float32).copy()
    w_qkv[:, :DIM] *= A_SCH * SCALE          # Schraudolph pre-scale on Q
    wqkv_bf = w_qkv.astype(NPBF16)
    wproj_bf = np.asarray(w_proj, np.float32).astype(NPBF16)
    batT = np.ascontiguousarray(batch.transpose(0, 2, 1)).astype(NPBF16)
    m = np.asarray(custom_mask, np.float32)[0, 0]  # [N, N] 0/1
    ident = np.eye(128, dtype=np.float32).astype(NPBF16)
    in_maps = []
    for c in range(NCORES):
        rows = slice(c * NQ, (c + 1) * NQ)
        qrTc = np.ascontiguousarray(batch[:, rows, :].transpose(0, 2, 1)).astype(NPBF16)
        mT = np.ascontiguousarray(m[rows, :].T)   # [N, NQ]
        mb = np.where(mT > 0, float(BC), MASK_NEG).astype(np.float32).astype(NPBF16)
        m01 = mT.astype(NPBF16)
        in_maps.append({
            "batt": batT, "qrt": qrTc, "wqkv": wqkv_bf,
            "wproj": wproj_bf, "maskb": mb, "mask01": m01, "ident": ident,
        })
    return in_maps


def _run(in_maps, trace=False, **kw):
    if "nc" not in _CACHE:
        _CACHE["nc"] = build_nc()
    return run_bass_kernel_spmd(
        _CACHE["nc"], in_maps, core_ids=list(range(NCORES)), trace=trace, **kw
    )


def kernel(batch, w_qkv, w_proj, custom_mask):
    in_maps = _prep_inputs(batch, w_qkv, w_proj, custom_mask)
    res = _run(in_maps)
    full = np.empty((B, N, DIM), np.float32)
    for c in range(NCORES):
        full[:, c * NQ:(c + 1) * NQ, :] = res.results[c]["out"]
    return full


# revision 14
# speedup vs baseline: 1.2935x; 1.0183x over previous
"""Sparse-attention Bass kernel for 8 TRN2 NeuronCores (Schraudolph split-route).

Sharding: query-row parallel. Core c owns query rows [c*512, (c+1)*512) of
both batch elements. K/V computed redundantly per core (no collectives).

The softmax exp is the bottleneck (67M elements/core must exit PSUM through
ACT or DVE at ~1 elem/cycle/lane). Two routes, statically interleaved:

  RC (~5/8 of score tiles): the PE pre-adds maskbias into the scores PSUM
     via an identity matmul (masked lanes get -30080 -> exp underflows to 0),
     then ONE ACT Exp instruction exits PSUM->SBUF bf16. Zero DVE cost.
  RB (~3/8): ONE DVE tensor_tensor (scores_f32 + maskbias_bf16 -> int16,
     round-to-nearest) computes a Schraudolph bit-trick exp: the int16 IS
     the bf16 bit pattern of exp(s)*2.41 (Q pre-scaled by 128*log2(e)*scale
     on the host). A second tensor (em2 = em1+64 bits) and a second
     accumulating AV matmul implement a two-point correction that cancels
     the piecewise-linear error (CV 1.78% -> 0.56%).

Rowsums come from M=1 ones-matmuls packed in tile_position col-bands, with
per-route ones weights (1.0 vs rho=2.414) so both routes carry the same
scale; the 2pt rowsum uses the constant-ratio approximation (row-to-row
variance ~0.04%).
"""

import numpy as np
from contextlib import ExitStack

import concourse.bass as bass
import concourse.tile as tile
from concourse import bacc, mybir
from concourse.bass_utils import run_bass_kernel_spmd

BF16 = mybir.dt.bfloat16
F32 = mybir.dt.float32
I16 = mybir.dt.int16
NPBF16 = mybir.dt.np(BF16)

B, N, DIM, H, D = 2, 4096, 512, 16, 32
NCORES = 8
NQ = N // NCORES
G = 4                       # head groups (4 heads each)
HG = H // G
JB = N // 128               # key blocks per batch elem (32)
SCALE = float(D) ** -0.5

# Schraudolph constants
A_SCH = 128.0 / float(np.log(2.0))      # bits per natural-log unit
BC = 127 * 128                          # 16256, bf16 exponent bias in bits
MASK_NEG = -30080.0                     # masked maskbias value (exact bf16)
D2PT = 64                               # two-point offset (bits)
# mean ratios of the approximations vs true exp (computed offline)
MU1 = 1.040683                          # solo em1 mean ratio
MU2 = 2.512434                          # em1 + em2 mean ratio (w=1, D=64)
RHO = MU2 / MU1                         # RB rowsum ones weight
LN2_128 = float(np.log(2.0)) / 128.0
# ACT exp bias: cancel the +BC carried in maskbias, add ln(MU2) to match RB
ACT_BIAS = -BC * LN2_128 + float(np.log(MU1))

# Route pattern: tile t is RB iff (t*3) % 8 < 3  -> 3/8 RB, 5/8 RC
def is_rb(t):
    return (t * 29) % 64 < 29

# Among RC tiles, a small subset uses the PE identity-matmul mask
# (relieves DVE); the rest use a DVE mask-multiply.
def is_idmm(t):
    return (t % 2) == 0

_CACHE = {}


def build_nc():
    nc = bacc.Bacc("TRN2", target_bir_lowering=False, debug=False)

    batT = nc.declare_dram_parameter("batt", [B, DIM, N], BF16, isOutput=False)
    qrT = nc.declare_dram_parameter("qrt", [B, DIM, NQ], BF16, isOutput=False)
    wqkv = nc.declare_dram_parameter("wqkv", [DIM, 3 * DIM], BF16, isOutput=False)
    wproj = nc.declare_dram_parameter("wproj", [DIM, DIM], BF16, isOutput=False)
    maskb = nc.declare_dram_parameter("maskb", [N, NQ], BF16, isOutput=False)
    mask01 = nc.declare_dram_parameter("mask01", [N, NQ], BF16, isOutput=False)
    ident = nc.declare_dram_parameter("ident", [128, 128], BF16, isOutput=False)
    out = nc.declare_dram_parameter("out", [B, NQ, DIM], F32, isOutput=True)

    Exp = mybir.ActivationFunctionType.Exp
    ADD = mybir.AluOpType.add

    with tile.TileContext(nc) as tc, ExitStack() as ctx:
        persist = ctx.enter_context(tc.tile_pool(name="persist", bufs=1))
        bpool = ctx.enter_context(tc.tile_pool(name="bpool", bufs=1))
        ktpool = ctx.enter_context(tc.tile_pool(name="ktpool", bufs=2))
        xpool = ctx.enter_context(tc.tile_pool(name="xpool", bufs=8))
        empool = ctx.enter_context(tc.tile_pool(name="empool", bufs=4))
        epool = ctx.enter_context(tc.tile_pool(name="epool", bufs=3))
        npool = ctx.enter_context(tc.tile_pool(name="npool", bufs=1))
        outp = ctx.enter_context(tc.tile_pool(name="outp", bufs=2))
        # PSUM: sc 3x2 banks + av 1 + rs 1 = 8
        scps = ctx.enter_context(tc.tile_pool(name="scps", bufs=3, space="PSUM"))
        avps = ctx.enter_context(tc.tile_pool(name="avps", bufs=1, space="PSUM"))
        rsps = ctx.enter_context(tc.tile_pool(name="rsps", bufs=1, space="PSUM"))

        # ---- persistent loads -------------------------------------------
        wq_sb = []
        for k in range(4):
            t = persist.tile([128, 3 * DIM], BF16, tag=f"wqkv{k}")
            nc.sync.dma_start(out=t, in_=wqkv[k * 128:(k + 1) * 128, :])
            wq_sb.append(t)
        wp_sb = []
        for k in range(4):
            t = persist.tile([128, DIM], BF16, tag=f"wproj{k}")
            nc.sync.dma_start(out=t, in_=wproj[k * 128:(k + 1) * 128, :])
            wp_sb.append(t)
        mb_sb = []
        for jb in range(JB):
            t = persist.tile([128, NQ], BF16, tag=f"mb{jb}")
            nc.sync.dma_start(out=t, in_=maskb[jb * 128:(jb + 1) * 128, :])
            mb_sb.append(t)
        m01_sb = []
        for jb in range(JB):
            t = persist.tile([128, NQ], BF16, tag=f"m01{jb}")
            nc.sync.dma_start(out=t, in_=mask01[jb * 128:(jb + 1) * 128, :])
            m01_sb.append(t)
        id_sb = persist.tile([128, 128], BF16, tag="ident")
        nc.sync.dma_start(out=id_sb, in_=ident[:, :])
        # ones weights for rowsum matmuls: cols 0..3 -> 1.0 (RC), 4..7 -> RHO (RB)
        ones_sb = persist.tile([128, 8], BF16, tag="ones")
        nc.vector.memset(ones_sb, 1.0)
        actbias = persist.tile([128, 1], F32, tag="actbias")
        nc.vector.memset(actbias, ACT_BIAS)
        actbias_p = persist.tile([128, 1], F32, tag="actbiasp")
        nc.vector.memset(actbias_p, float(np.log(MU1)))

        def mrep(mb_t):
            return bass.AP(
                tensor=mb_t.tensor, offset=mb_t.offset,
                ap=[mb_t.ap[0], [0, 2], [1, NQ]],
            )

        tile_ctr = [0]

        for b in range(B):
            # per-b SBUF
            qrT_sb = []
            for k in range(4):
                t = bpool.tile([128, NQ], BF16, tag=f"qrT{k}")
                nc.sync.dma_start(out=t, in_=qrT[b, k * 128:(k + 1) * 128, :])
                qrT_sb.append(t)
            v_sb = bpool.tile([128, JB * 512], BF16, tag="vall")
            qt_sb = [bpool.tile([128, NQ], BF16, tag=f"qt{g}", name=f"qt{g}")
                     for g in range(G)]

            # batT slices streamed: jc in 0..7, each [4k][128, 512]
            bat_tiles = {}

            def load_bat(jc):
                tl = []
                for k in range(4):
                    t = xpool.tile([128, 512], BF16, tag=f"bat{k}")
                    nc.sync.dma_start(
                        out=t, in_=batT[b, k * 128:(k + 1) * 128,
                                        jc * 512:(jc + 1) * 512])
                    tl.append(t)
                bat_tiles[jc] = tl

            def make_kt_pair(kt_t, g, jc2):
                # two adjacent 512-chunks of K^T(g) -> one [128,1024] psum
                # tile -> one ACT copy into kt_t[:, jc2*1024 : +1024]
                ps = scps.tile([128, 1024], F32, tag="sc")
                for half in range(2):
                    jc = jc2 * 2 + half
                    bt = bat_tiles[jc]
                    for k in range(4):
                        nc.tensor.matmul(
                            ps[:, half * 512:(half + 1) * 512],
                            wq_sb[k][:, DIM + 128 * g: DIM + 128 * (g + 1)],
                            bt[k],
                            start=(k == 0), stop=(k == 3),
                        )
                nc.scalar.copy(kt_t[:, jc2 * 1024:(jc2 + 1) * 1024], ps)

            def make_v_pair(nb2):
                # two adjacent 128-row V blocks -> [128,1024] psum -> v_sb
                ps = scps.tile([128, 1024], F32, tag="sc")
                for half in range(2):
                    nb = nb2 * 2 + half
                    jc = nb // 4
                    bt = bat_tiles[jc]
                    off = (nb % 4) * 128
                    for k in range(4):
                        nc.tensor.matmul(
                            ps[:, half * 512:(half + 1) * 512],
                            bt[k][:, off:off + 128],
                            wq_sb[k][:, 2 * DIM: 3 * DIM],
                            start=(k == 0), stop=(k == 3),
                        )
                nc.scalar.copy(v_sb[:, nb2 * 1024:(nb2 + 1) * 1024], ps)

            def make_qt(g):
                ps = scps.tile([128, 1024], F32, tag="sc")
                for k in range(4):
                    nc.tensor.matmul(
                        ps[:, 0:512],
                        wq_sb[k][:, 128 * g: 128 * (g + 1)],
                        qrT_sb[k],
                        start=(k == 0), stop=(k == 3),
                    )
                nc.scalar.copy(qt_sb[g], ps[:, 0:512])

            # ---- preamble: batT jc=0..7 streamed in, kt[0], qt[0], v 0..7
            kt_cur = ktpool.tile([128, N], BF16, tag="kt")
            for jc in range(8):
                load_bat(jc)
            make_qt(0)
            for jc2 in range(4):
                make_kt_pair(kt_cur, 0, jc2)
            for nb2 in range(4):
                make_v_pair(nb2)

            # deferred work lists per g: list of callables
            kt_next = [None]
            def defer_for_g(g):
                work = []
                if g == 0:
                    for nb2 in range(4, 16):
                        work.append(lambda nb2=nb2: make_v_pair(nb2))
                if g < G - 1:
                    nk = ktpool.tile([128, N], BF16, tag="kt")
                    kt_next[0] = nk
                    for jc2 in range(4):
                        work.append(lambda nk=nk, g1=g + 1, jc2=jc2:
                                    make_kt_pair(nk, g1, jc2))
                    work.append(lambda g1=g + 1: make_qt(g1))
                return work

            pre_sb = [bpool.tile([128, NQ], BF16, tag=f"pre{g}", name=f"pre{g}")
                      for g in range(G)]

            for g in range(G):
                work = defer_for_g(g)
                wi = 0
                av = avps.tile([128, NQ], F32, tag="av")
                rs = rsps.tile([128, NQ], F32, tag="rs")
                av_started = [False] * HG
                rs_started = [False] * HG

                def emit_waves(em_halves, wjb, last):
                    vsl = v_sb[:, wjb * 512:(wjb + 1) * 512]
                    for r in range(HG):
                        h1 = em_halves[r]
                        hh = g * HG + r
                        nc.tensor.matmul(
                            av[32 * r:32 * r + 32, :],
                            vsl[:, 32 * hh:32 * hh + 32], h1,
                            start=not av_started[r], stop=last,
                            tile_position=(0, 32 * r),
                        )
                        av_started[r] = True
                    for r in range(HG):
                        h1 = em_halves[r]
                        nc.tensor.matmul(
                            rs[32 * r:32 * r + 1, :],
                            ones_sb[:, r:r + 1], h1,
                            start=not rs_started[r], stop=last,
                            tile_position=(0, 32 * r),
                        )
                        rs_started[r] = True

                pending = None
                for jb in range(JB):
                    # interleave deferred QKV work (g0 has 17 items: 1/iter)
                    if wi < len(work) and (g == 0 or jb % 2 == 0):
                        work[wi](); wi += 1

                    mb_t = mb_sb[jb]
                    em_halves = []
                    for pair in range(2):     # heads (0,1) then (2,3)
                        t_idx = tile_ctr[0]; tile_ctr[0] += 1
                        rb = is_rb(t_idx)
                        idm = (not rb) and is_idmm(t_idx)
                        sc = scps.tile([128, 1024], F32, tag="sc")
                        for r2 in range(2):
                            r = pair * 2 + r2
                            nc.tensor.matmul(
                                sc[:, r2 * 512:(r2 + 1) * 512],
                                kt_cur[32 * r:32 * r + 32, jb * 128:(jb + 1) * 128],
                                qt_sb[g][32 * r:32 * r + 32, :],
                                start=True, stop=not idm,
                                tile_position=(32 * r, 0),
                            )
                        if rb:
                            em1 = empool.tile([128, 1024], I16, tag="em1")
                            nc.vector.tensor_tensor(em1, sc, mrep(mb_t), ADD)
                            e1b = em1.bitcast(BF16)
                            for r2 in range(2):
                                em_halves.append(e1b[:, r2 * 512:(r2 + 1) * 512])
                        elif idm:
                            for r2 in range(2):
                                nc.tensor.matmul(
                                    sc[:, r2 * 512:(r2 + 1) * 512],
                                    id_sb, mb_t,
                                    start=False, stop=True,
                                )
                            e = epool.tile([128, 1024], BF16, tag="e")
                            nc.scalar.activation(e, sc, Exp,
                                                 scale=LN2_128, bias=actbias)
                            for r2 in range(2):
                                em_halves.append(e[:, r2 * 512:(r2 + 1) * 512])
                        else:
                            eraw = epool.tile([128, 1024], BF16, tag="eraw")
                            nc.scalar.activation(eraw, sc, Exp,
                                                 scale=LN2_128, bias=actbias_p)
                            e = epool.tile([128, 1024], BF16, tag="e")
                            nc.vector.tensor_mul(e, eraw, mrep(m01_sb[jb]))
                            for r2 in range(2):
                                em_halves.append(e[:, r2 * 512:(r2 + 1) * 512])

                    if pending is not None:
                        emit_waves(pending[0], pending[1], last=False)
                    pending = (em_halves, jb)
                emit_waves(pending[0], pending[1], last=True)

                # ---- normalize group g (baseline-proven pattern) --------
                for r in range(HG):
                    rsr = npool.tile([1, NQ], F32, tag="rsr")
                    nc.vector.tensor_copy(rsr, rs[32 * r:32 * r + 1, :])
                    rcp = npool.tile([1, NQ], F32, tag="rcp")
                    nc.vector.reciprocal_approx_fast(rcp, rsr)
                    rcpb = npool.tile([32, NQ], F32, tag="rcpb")
                    nc.gpsimd.partition_broadcast(rcpb, rcp[0:1, :], channels=32)
                    nc.vector.tensor_mul(
                        pre_sb[g][32 * r:32 * r + 32, :],
                        av[32 * r:32 * r + 32, :],
                        rcpb,
                    )

                if g < G - 1:
                    kt_cur = kt_next[0]

            # ---- output projection --------------------------------------
            for ib in range(NQ // 128):
                ps = rsps.tile([128, NQ], F32, tag="rs")
                for g in range(G):
                    nc.tensor.matmul(
                        ps[:, 0:DIM],
                        pre_sb[g][:, ib * 128:(ib + 1) * 128],
                        wp_sb[g],
                        start=(g == 0), stop=(g == 3),
                    )
                o = outp.tile([128, DIM], F32, tag="o")
                nc.scalar.copy(o, ps[:, 0:DIM])
                nc.sync.dma_start(out=out[b, ib * 128:(ib + 1) * 128, :], in_=o)

    nc.compile()
    return nc


def _prep_inputs(batch, w_qkv, w_proj, custom_mask):
    batch = np.asarray(batch, np.float32)
    w_qkv = np.asarray(w_qkv, np.float32).copy()
    w_qkv[:, :DIM] *= A_SCH * SCALE          # Schraudolph pre-scale on Q
    wqkv_bf = w_qkv.astype(NPBF16)
    wproj_bf = np.asarray(w_proj, np.float32).astype(NPBF16)
    batT = np.ascontiguousarray(batch.transpose(0, 2, 1)).astype(NPBF16)
    m = np.asarray(custom_mask, np.float32)[0, 0]  # [N, N] 0/1
    ident = np.eye(128, dtype=np.float32).astype(NPBF16)
    in_maps = []
    for c in range(NCORES):
        rows = slice(c * NQ, (c + 1) * NQ)
        qrTc = np.ascontiguousarray(batch[:, rows, :].transpose(0, 2, 1)).astype(NPBF16)
        mT = np.ascontiguousarray(m[rows, :].T)   # [N, NQ]
        mb = np.where(mT > 0, float(BC), MASK_NEG).astype(np.float32).astype(NPBF16)
        m01 = mT.astype(NPBF16)
        in_maps.append({
            "batt": batT, "qrt": qrTc, "wqkv": wqkv_bf,
            "wproj": wproj_bf, "maskb": mb, "mask01": m01, "ident": ident,
        })
    return in_maps


def _run(in_maps, trace=False, **kw):
    if "nc" not in _CACHE:
        _CACHE["nc"] = build_nc()
    return run_bass_kernel_spmd(
        _CACHE["nc"], in_maps, core_ids=list(range(NCORES)), trace=trace, **kw
    )


def kernel(batch, w_qkv, w_proj, custom_mask):
    in_maps = _prep_inputs(batch, w_qkv, w_proj, custom_mask)
    res = _run(in_maps)
    full = np.empty((B, N, DIM), np.float32)
    for c in range(NCORES):
        full[:, c * NQ:(c + 1) * NQ, :] = res.results[c]["out"]
    return full
